# revision 8
# baseline (speedup 1.0000x reference)
"""LoRALinear (paged multi-adapter LoRA + base linear) Trainium2 kernel.

Full-input contract: kernel(**inputs) takes the unsharded tensors and
returns the full [T, D_OUT] output.

Sharding: tokens are split contiguously across the 8 NeuronCores
(1024 tokens/core).  The base weight, bias and the (tiny) LoRA page
caches are preprocessed on host into per-core dense operands:

  out_c = x_c @ W^T + bias + ((x_c @ A_c^T) * mask_c) @ B_c

All matmul operands are fp16 (rel err ~3e-4 vs the 2e-2 gate), which
runs the PE at the same 1 row/cycle as fp32r but halves weight-load
time and all DMA traffic.  PSUM accumulation stays fp32.  The bias is
folded into the LoRA-B matmul as one extra contraction row.

Device schedule (per core):
  - x^T (+ the 64 LoRA-A rows, fused into the same per-k slab DMA)
    stays resident in SBUF (~9 MiB fp16).
  - W is host-packed so each DMA is a [128, 2048] fp16 "quad" (4 KiB
    per partition row) holding 4 k-tiles of one n-block.
  - n=0's 8 quads stay resident, so the two m-tiles displaced by the
    LoRA-A PSUM accumulators during startup run right after the mask
    phase with zero extra DMA (the baseline re-streamed 8 MiB at the
    very end).
  - Output staging DMAs ride the scalar engine's queue and are split
    across DGE queues (the final tile's 256 KiB on one ~19 GB/s queue
    was a 12 us tail in the baseline).
"""

import os

import numpy as np

import concourse.bass as bass
import concourse.bacc as bacc
import concourse.mybir as mybir
import concourse.tile as tile
from concourse.bass_utils import run_bass_kernel_spmd

N_CORES = 8
T = 8192
D_IN = 4096
D_OUT = 4096
TPC = T // N_CORES  # tokens per core
MAX_RANK = 64
P = 128
NFREE = 512  # matmul moving free dim (psum bank)
KT = D_IN // P          # 32 k tiles
MT = TPC // P           # 8 token (m) tiles
NT = D_OUT // NFREE     # 8 output (n) blocks
QK = 4                  # k tiles per packed W quad
NQ = KT // QK           # 8 quads per n block
XAW = TPC + MAX_RANK    # fused x+A slab width (1088)

F32 = mybir.dt.float32
F16 = mybir.dt.float16

# exec time of the last device run (ns), when KERNEL_TRACE=1
last_exec_time_ns = None
last_results = None


def _build_program(r_aug=MAX_RANK + 1, d_out=D_OUT, tpc=TPC):
    """Build the per-core Bass program (G=1: 64 LoRA rows + bias row)."""
    t_chunks = tpc // NFREE  # 2
    m_inline = MT - t_chunks  # 6 m-tiles of n=0 run during startup
    defer = list(range(m_inline, MT))

    nc = bacc.Bacc("TRN2", target_bir_lowering=False, debug=False)

    xaT = nc.dram_tensor("xaT", [D_IN, XAW], F16, kind="ExternalInput").ap()
    wP = nc.dram_tensor("wP", [NQ * P, NT * QK * NFREE], F16,
                        kind="ExternalInput").ap()
    bS = nc.dram_tensor("bS", [r_aug, d_out], F16, kind="ExternalInput").ap()
    mS = nc.dram_tensor("mS", [r_aug, tpc], F32, kind="ExternalInput").ap()
    out = nc.dram_tensor("out", [tpc, d_out], F32, kind="ExternalOutput").ap()

    QW = QK * NFREE  # 2048 cols per quad

    with tile.TileContext(nc) as tc:
        with (
            tc.tile_pool(name="xapool", bufs=KT) as xapool,
            tc.tile_pool(name="w0pool", bufs=NQ) as w0pool,
            tc.tile_pool(name="cpool", bufs=1) as cpool,
            tc.tile_pool(name="wpool", bufs=6) as wpool,
            tc.tile_pool(name="opool", bufs=6) as opool,
            tc.tile_pool(name="psum", bufs=8, space="PSUM") as psum,
        ):
            # ---- DMA issue order: first k slab + first W quad lead ----
            # every big load is split along partitions so consecutive
            # dma_starts land on different DGE queues (~30 GB/s each)
            xs = []
            w0 = []

            def split_dma(eng, dst, src, parts):
                rows = dst.partition_size() // parts
                for i in range(parts):
                    rs = slice(i * rows, (i + 1) * rows)
                    eng.dma_start(dst[rs, :], src[rs, :])

            def xa_dma(k, parts=2):
                t = xapool.tile([P, XAW], F16, tag="xa", name=f"xa_{k}")
                split_dma(nc.sync, t, xaT[k * P:(k + 1) * P, :], parts)
                xs.append(t)

            def w0_dma(j):
                t = w0pool.tile([P, QW], F16, tag="w0", name=f"w0_{j}")
                split_dma(nc.sync, t, wP[j * P:(j + 1) * P, 0:QW], 2)
                w0.append(t)

            xa_dma(0, parts=4)
            w0_dma(0)
            xa_dma(1)
            bss = cpool.tile([r_aug, d_out], F16, tag="bss", name="bss")
            nc.sync.dma_start(bss, bS)
            mss = cpool.tile([r_aug, tpc], F32, tag="mss", name="mss")
            nc.sync.dma_start(mss, mS)
            xam = cpool.tile([r_aug, tpc], F16, tag="xam", name="xam")
            w0_dma(1)
            for k in range(2, KT):
                xa_dma(k)
                j = k // QK + 1  # quad prefetch ~1 quad ahead of use
                if k % QK == 0 and j < NQ:
                    w0_dma(j)

            # ones row for the bias contraction (mss row 64 is all 1.0)
            nc.vector.tensor_copy(xam[MAX_RANK:r_aug, :],
                                  mss[MAX_RANK:r_aug, :])

            # ---- PE startup: per k, LoRA-A + 6 inline n=0 m-tiles ----
            lora_ps = [psum.tile([MAX_RANK, NFREE], F32, tag="ps",
                                 name=f"ps_lora_{c}") for c in range(t_chunks)]
            psts0 = [psum.tile([P, NFREE], F32, tag="ps", name=f"pst0_{m}")
                     for m in range(m_inline)]
            for k in range(KT):
                wq = w0[k // QK]
                qs = (k % QK) * NFREE
                for c in range(t_chunks):
                    nc.tensor.matmul(
                        lora_ps[c],
                        lhsT=xs[k][:, tpc:tpc + MAX_RANK],
                        rhs=xs[k][:, c * NFREE:(c + 1) * NFREE],
                        start=(k == 0),
                        stop=(k == KT - 1),
                    )
                for m in range(m_inline):
                    nc.tensor.matmul(
                        psts0[m],
                        lhsT=xs[k][:, m * P:(m + 1) * P],
                        rhs=wq[:, qs:qs + NFREE],
                        start=(k == 0),
                        stop=False,
                    )

            # ---- masks: xam = lora_ps * mS (frees the two LoRA banks) ----
            for c in range(t_chunks):
                tsl = slice(c * NFREE, (c + 1) * NFREE)
                nc.vector.tensor_mul(xam[0:MAX_RANK, tsl],
                                     lora_ps[c],
                                     mss[0:MAX_RANK, tsl])

            def lora_b(pst, m, nsl):
                nc.tensor.matmul(
                    pst,
                    lhsT=xam[:, m * P:(m + 1) * P],
                    rhs=bss[:, nsl],
                    start=False,
                    stop=True,
                )

            def copy_out(m, n, pst, parts=2):
                ot = opool.tile([P, NFREE], F32, tag="ot", name=f"ot_{n}_{m}")
                nc.vector.tensor_copy(ot, pst)
                # scalar engine's DGE queue, split so rows land on several
                # queues (a single 256 KiB store drains at ~19 GB/s)
                rows = P // parts
                for i in range(parts):
                    nc.scalar.dma_start(
                        out[m * P + i * rows:m * P + (i + 1) * rows,
                            n * NFREE:(n + 1) * NFREE],
                        ot[i * rows:(i + 1) * rows, :])

            # ---- n=0 completion for the inline m-tiles ----
            n0sl = slice(0, NFREE)
            for m in range(m_inline):
                lora_b(psts0[m], m, n0sl)
                copy_out(m, 0, psts0[m])

            # ---- steady state: n = 1..7 ----
            for n in range(1, NT):
                nsl = slice(n * NFREE, (n + 1) * NFREE)
                psts = [psum.tile([P, NFREE], F32, tag="ps",
                                  name=f"pst_{n}_{m}") for m in range(MT)]
                for j in range(NQ):
                    wq = wpool.tile([P, QW], F16, tag="wt", name=f"wt_{n}_{j}")
                    split_dma(nc.sync, wq,
                              wP[j * P:(j + 1) * P, n * QW:(n + 1) * QW], 2)
                    for q in range(QK):
                        k = j * QK + q
                        for m in range(MT):
                            nc.tensor.matmul(
                                psts[m],
                                lhsT=xs[k][:, m * P:(m + 1) * P],
                                rhs=wq[:, q * NFREE:(q + 1) * NFREE],
                                start=(k == 0),
                                stop=False,
                            )
                for m in range(MT):
                    lora_b(psts[m], m, nsl)
                    copy_out(m, n, psts[m])

            # ---- tail: the two n=0 m-tiles displaced by the LoRA-A psum
            # accumulators, straight from the resident quads (no DMA in,
            # only 512 KiB out -> short drain) ----
            pstd = [psum.tile([P, NFREE], F32, tag="ps", name=f"pstd_{m}")
                    for m in defer]
            for k in range(KT):
                wq = w0[k // QK]
                qs = (k % QK) * NFREE
                for i, m in enumerate(defer):
                    nc.tensor.matmul(
                        pstd[i],
                        lhsT=xs[k][:, m * P:(m + 1) * P],
                        rhs=wq[:, qs:qs + NFREE],
                        start=(k == 0),
                        stop=False,
                    )
            for i, m in enumerate(defer):
                lora_b(pstd[i], m, n0sl)
                copy_out(m, 0, pstd[i], parts=4)

    nc.compile()
    return nc


def _prep_core_inputs(x16, weight_p, bias, a_cache, b_cache, tok_adapter,
                      tok_scale, rank_page_table, ranks, core):
    """Host-side shard prep for one core (single adapter per core)."""
    d_out = b_cache.shape[1]
    r_aug = MAX_RANK + 1
    sl = slice(core * TPC, (core + 1) * TPC)
    adapters = tok_adapter[sl]
    scales = tok_scale[sl]
    a = int(adapters[0])
    pages = rank_page_table[a]  # [64] page ids

    xaT = np.empty((D_IN, XAW), np.float16)
    xaT[:, :TPC] = x16[sl].T
    xaT[:, TPC:] = a_cache[pages].T.astype(np.float16)

    bS = np.empty((r_aug, d_out), np.float16)
    bS[:MAX_RANK] = b_cache[pages].astype(np.float16)
    bS[MAX_RANK] = bias.astype(np.float16)

    mS = np.empty((r_aug, TPC), np.float32)
    slot_active = (np.arange(MAX_RANK) < ranks[a])[:, None]  # [64, 1]
    tok_active = (adapters == a)[None, :]  # [1, TPC]
    mS[:MAX_RANK] = (slot_active & tok_active) * scales[None, :]
    mS[MAX_RANK] = 1.0
    return {"xaT": xaT, "wP": weight_p, "bS": bS, "mS": mS}


def kernel(x, weight, bias, a_cache, b_cache, b_start_loc, b_adapter_ids,
           b_scaling, rank_page_table, ranks):
    global last_exec_time_ns, last_results
    x = np.asarray(x, np.float32)
    weight = np.asarray(weight, np.float32)
    bias = np.asarray(bias, np.float32)
    a_cache = np.asarray(a_cache, np.float32)
    b_cache = np.asarray(b_cache, np.float32)
    b_start_loc = np.asarray(b_start_loc)
    b_adapter_ids = np.asarray(b_adapter_ids)
    b_scaling = np.asarray(b_scaling, np.float32)
    rank_page_table = np.asarray(rank_page_table)
    ranks = np.asarray(ranks)

    t = x.shape[0]
    seg = np.searchsorted(b_start_loc, np.arange(t, dtype=b_start_loc.dtype),
                          side="right") - 1
    tok_adapter = b_adapter_ids[seg]
    tok_scale = b_scaling[seg]
    # this schedule assumes one adapter per 1024-token core slice (the
    # spec's equal-length-sequence layout guarantees it)
    assert all(
        len(np.unique(tok_adapter[c * TPC:(c + 1) * TPC])) == 1
        for c in range(N_CORES)
    )

    x16 = x.astype(np.float16)
    # pack W^T into [NQ*128, NT*2048]: row j*128+p, col n*2048 + q*512 + c
    # holds W^T[(4j+q)*128 + p, n*512 + c]
    wt = np.ascontiguousarray(weight.T.astype(np.float16))
    weight_p = np.ascontiguousarray(
        wt.reshape(NQ, QK, P, NT, NFREE)
          .transpose(0, 2, 3, 1, 4)
          .reshape(NQ * P, NT * QK * NFREE))

    in_maps = [
        _prep_core_inputs(x16, weight_p, bias, a_cache, b_cache, tok_adapter,
                          tok_scale, rank_page_table, ranks, c)
        for c in range(N_CORES)
    ]

    nc = _build_program()
    trace = os.environ.get("KERNEL_TRACE", "0") == "1"
    repeat = int(os.environ.get("KERNEL_REPEAT", "1"))
    times = []
    for _ in range(repeat):
        res = run_bass_kernel_spmd(nc, in_maps, core_ids=list(range(N_CORES)),
                                   trace=trace)
        times.append(res.exec_time_ns)
    last_exec_time_ns = (min(t for t in times if t is not None)
                         if any(t is not None for t in times) else None)
    last_results = res
    if repeat > 1:
        print("exec times:", times)
    return np.concatenate([res.results[c]["out"] for c in range(N_CORES)],
                          axis=0).astype(np.float32)


# revision 12
# speedup vs baseline: 1.0694x; 1.0694x over previous
"""LoRALinear (paged multi-adapter LoRA + base linear) Trainium2 kernel.

Full-input contract: kernel(**inputs) takes the unsharded tensors and
returns the full [T, D_OUT] output.

Sharding: tokens are split contiguously across the 8 NeuronCores
(1024 tokens/core).  The base weight, bias and the (tiny) LoRA page
caches are preprocessed on host into per-core dense operands:

  out_c = x_c @ W^T + bias + ((x_c @ A_c^T) * mask_c) @ B_c

All matmul operands are fp16 (rel err ~3e-4 vs the 2e-2 gate), which
runs the PE at the same 1 row/cycle as fp32r but halves weight-load
time and all DMA traffic.  PSUM accumulation stays fp32.  The bias is
folded into the LoRA-B matmul as one extra contraction row.

Device schedule (per core):
  - x^T (+ the 64 LoRA-A rows, fused into the same per-k slab DMA)
    stays resident in SBUF (~9 MiB fp16).
  - W is host-packed so each DMA is a [128, 2048] fp16 "quad" (4 KiB
    per partition row) holding 4 k-tiles of one n-block.
  - n=0's 8 quads stay resident, so the two m-tiles displaced by the
    LoRA-A PSUM accumulators during startup run right after the mask
    phase with zero extra DMA (the baseline re-streamed 8 MiB at the
    very end).
  - Output staging DMAs ride the scalar engine's queue and are split
    across DGE queues (the final tile's 256 KiB on one ~19 GB/s queue
    was a 12 us tail in the baseline).
"""

import os

import numpy as np

import concourse.bass as bass
import concourse.bacc as bacc
import concourse.mybir as mybir
import concourse.tile as tile
from concourse.bass_utils import run_bass_kernel_spmd

N_CORES = 8
T = 8192
D_IN = 4096
D_OUT = 4096
TPC = T // N_CORES  # tokens per core
MAX_RANK = 64
P = 128
NFREE = 512  # matmul moving free dim (psum bank)
KT = D_IN // P          # 32 k tiles
MT = TPC // P           # 8 token (m) tiles
NT = D_OUT // NFREE     # 8 output (n) blocks
QK = 4                  # k tiles per packed W quad
NQ = KT // QK           # 8 quads per n block
XAW = TPC + MAX_RANK    # fused x+A slab width (1088)

F32 = mybir.dt.float32
F16 = mybir.dt.float16

# exec time of the last device run (ns), when KERNEL_TRACE=1
last_exec_time_ns = None
last_results = None


def _build_program(r_aug=MAX_RANK + 1, d_out=D_OUT, tpc=TPC):
    """Build the per-core Bass program (G=1: 64 LoRA rows + bias row)."""
    t_chunks = tpc // NFREE  # 2
    m_inline = MT - t_chunks  # 6 m-tiles of n=0 run during startup
    defer = list(range(m_inline, MT))

    nc = bacc.Bacc("TRN2", target_bir_lowering=False, debug=False)

    xaT = nc.dram_tensor("xaT", [D_IN, XAW], F16, kind="ExternalInput").ap()
    wP = nc.dram_tensor("wP", [NQ * P, NT * QK * NFREE], F16,
                        kind="ExternalInput").ap()
    bS = nc.dram_tensor("bS", [r_aug, d_out], F16, kind="ExternalInput").ap()
    mS = nc.dram_tensor("mS", [r_aug, tpc], F32, kind="ExternalInput").ap()
    out = nc.dram_tensor("out", [tpc, d_out], F32, kind="ExternalOutput").ap()

    QW = QK * NFREE  # 2048 cols per quad

    with tile.TileContext(nc) as tc:
        with (
            tc.tile_pool(name="xapool", bufs=KT) as xapool,
            tc.tile_pool(name="w0pool", bufs=NQ) as w0pool,
            tc.tile_pool(name="cpool", bufs=1) as cpool,
            tc.tile_pool(name="wpool", bufs=6) as wpool,
            tc.tile_pool(name="opool", bufs=6) as opool,
            tc.tile_pool(name="psum", bufs=8, space="PSUM") as psum,
        ):
            # ---- DMA issue order: first k slab + first W quad lead ----
            # every big load is split along partitions so consecutive
            # dma_starts land on different DGE queues (~30 GB/s each)
            xs = []
            w0 = []

            def split_dma(eng, dst, src, parts):
                n_rows = dst.partition_size()
                rows = n_rows // parts
                for i in range(parts):
                    rs = slice(i * rows,
                               (i + 1) * rows if i < parts - 1 else n_rows)
                    eng.dma_start(dst[rs, :], src[rs, :])

            def xa_dma(k, parts=2):
                t = xapool.tile([P, XAW], F16, tag="xa", name=f"xa_{k}")
                split_dma(nc.sync, t, xaT[k * P:(k + 1) * P, :], parts)
                xs.append(t)

            def w0_dma(j, parts=2):
                t = w0pool.tile([P, QW], F16, tag="w0", name=f"w0_{j}")
                split_dma(nc.sync, t, wP[j * P:(j + 1) * P, 0:QW], parts)
                w0.append(t)

            xa_dma(0, parts=4)
            w0_dma(0, parts=4)
            xa_dma(1)
            w0_dma(1, parts=4)
            bss = cpool.tile([r_aug, d_out], F16, tag="bss", name="bss")
            mss = cpool.tile([r_aug, tpc], F32, tag="mss", name="mss")
            xam = cpool.tile([r_aug, tpc], F16, tag="xam", name="xam")
            for k in range(2, KT):
                xa_dma(k)
                j = k // QK + 1  # quad prefetch ~1 quad ahead of use
                if k % QK == 0 and j < NQ:
                    w0_dma(j)
                if k == 8:
                    # small mid-flight loads, only needed at mask time
                    split_dma(nc.sync, bss, bS, 2)
                    nc.sync.dma_start(mss, mS)

            # ones row for the bias contraction (mss row 64 is all 1.0)
            nc.vector.tensor_copy(xam[MAX_RANK:r_aug, :],
                                  mss[MAX_RANK:r_aug, :])

            # ---- PE startup: per k, LoRA-A + 6 inline n=0 m-tiles ----
            lora_ps = [psum.tile([MAX_RANK, NFREE], F32, tag="ps",
                                 name=f"ps_lora_{c}") for c in range(t_chunks)]
            psts0 = [psum.tile([P, NFREE], F32, tag="ps", name=f"pst0_{m}")
                     for m in range(m_inline)]
            for k in range(KT):
                wq = w0[k // QK]
                qs = (k % QK) * NFREE
                for c in range(t_chunks):
                    nc.tensor.matmul(
                        lora_ps[c],
                        lhsT=xs[k][:, tpc:tpc + MAX_RANK],
                        rhs=xs[k][:, c * NFREE:(c + 1) * NFREE],
                        start=(k == 0),
                        stop=(k == KT - 1),
                    )
                for m in range(m_inline):
                    nc.tensor.matmul(
                        psts0[m],
                        lhsT=xs[k][:, m * P:(m + 1) * P],
                        rhs=wq[:, qs:qs + NFREE],
                        start=(k == 0),
                        stop=False,
                    )

            # ---- masks: xam = lora_ps * mS (frees the two LoRA banks) ----
            for c in range(t_chunks):
                tsl = slice(c * NFREE, (c + 1) * NFREE)
                nc.vector.tensor_mul(xam[0:MAX_RANK, tsl],
                                     lora_ps[c],
                                     mss[0:MAX_RANK, tsl])

            def lora_b(pst, m, nsl):
                nc.tensor.matmul(
                    pst,
                    lhsT=xam[:, m * P:(m + 1) * P],
                    rhs=bss[:, nsl],
                    start=False,
                    stop=True,
                )

            def copy_out(m, n, pst, parts=2):
                ot = opool.tile([P, NFREE], F32, tag="ot", name=f"ot_{n}_{m}")
                nc.vector.tensor_copy(ot, pst)
                # scalar engine's DGE queue, split so rows land on several
                # queues (a single 256 KiB store drains at ~19 GB/s)
                rows = P // parts
                for i in range(parts):
                    nc.scalar.dma_start(
                        out[m * P + i * rows:m * P + (i + 1) * rows,
                            n * NFREE:(n + 1) * NFREE],
                        ot[i * rows:(i + 1) * rows, :])

            # ---- n=0 completion for the inline m-tiles ----
            n0sl = slice(0, NFREE)
            for m in range(m_inline):
                lora_b(psts0[m], m, n0sl)
                copy_out(m, 0, psts0[m])

            # ---- steady state: n = 1..7 ----
            for n in range(1, NT):
                nsl = slice(n * NFREE, (n + 1) * NFREE)
                psts = [psum.tile([P, NFREE], F32, tag="ps",
                                  name=f"pst_{n}_{m}") for m in range(MT)]
                for j in range(NQ):
                    wq = wpool.tile([P, QW], F16, tag="wt", name=f"wt_{n}_{j}")
                    split_dma(nc.sync, wq,
                              wP[j * P:(j + 1) * P, n * QW:(n + 1) * QW], 2)
                    for q in range(QK):
                        k = j * QK + q
                        for m in range(MT):
                            nc.tensor.matmul(
                                psts[m],
                                lhsT=xs[k][:, m * P:(m + 1) * P],
                                rhs=wq[:, q * NFREE:(q + 1) * NFREE],
                                start=(k == 0),
                                stop=False,
                            )
                            if k == KT - 1:
                                # finish tile m right away: the psum bank
                                # frees while the PE streams m+1..7, so the
                                # next n-block starts without a copy stall
                                lora_b(psts[m], m, nsl)
                                copy_out(m, n, psts[m])

            # ---- tail: the two n=0 m-tiles displaced by the LoRA-A psum
            # accumulators, straight from the resident quads (no DMA in,
            # only 512 KiB out -> short drain) ----
            pstd = [psum.tile([P, NFREE], F32, tag="ps", name=f"pstd_{m}")
                    for m in defer]
            for k in range(KT):
                wq = w0[k // QK]
                qs = (k % QK) * NFREE
                for i, m in enumerate(defer):
                    nc.tensor.matmul(
                        pstd[i],
                        lhsT=xs[k][:, m * P:(m + 1) * P],
                        rhs=wq[:, qs:qs + NFREE],
                        start=(k == 0),
                        stop=False,
                    )
                    if k == KT - 1:
                        lora_b(pstd[i], m, n0sl)
                        copy_out(m, 0, pstd[i])

    nc.compile()
    return nc


def _prep_core_inputs(x16, weight_p, bias, a_cache, b_cache, tok_adapter,
                      tok_scale, rank_page_table, ranks, core):
    """Host-side shard prep for one core (single adapter per core)."""
    d_out = b_cache.shape[1]
    r_aug = MAX_RANK + 1
    sl = slice(core * TPC, (core + 1) * TPC)
    adapters = tok_adapter[sl]
    scales = tok_scale[sl]
    a = int(adapters[0])
    pages = rank_page_table[a]  # [64] page ids

    xaT = np.empty((D_IN, XAW), np.float16)
    xaT[:, :TPC] = x16[sl].T
    xaT[:, TPC:] = a_cache[pages].T.astype(np.float16)

    bS = np.empty((r_aug, d_out), np.float16)
    bS[:MAX_RANK] = b_cache[pages].astype(np.float16)
    bS[MAX_RANK] = bias.astype(np.float16)

    mS = np.empty((r_aug, TPC), np.float32)
    slot_active = (np.arange(MAX_RANK) < ranks[a])[:, None]  # [64, 1]
    tok_active = (adapters == a)[None, :]  # [1, TPC]
    mS[:MAX_RANK] = (slot_active & tok_active) * scales[None, :]
    mS[MAX_RANK] = 1.0
    return {"xaT": xaT, "wP": weight_p, "bS": bS, "mS": mS}


def kernel(x, weight, bias, a_cache, b_cache, b_start_loc, b_adapter_ids,
           b_scaling, rank_page_table, ranks):
    global last_exec_time_ns, last_results
    x = np.asarray(x, np.float32)
    weight = np.asarray(weight, np.float32)
    bias = np.asarray(bias, np.float32)
    a_cache = np.asarray(a_cache, np.float32)
    b_cache = np.asarray(b_cache, np.float32)
    b_start_loc = np.asarray(b_start_loc)
    b_adapter_ids = np.asarray(b_adapter_ids)
    b_scaling = np.asarray(b_scaling, np.float32)
    rank_page_table = np.asarray(rank_page_table)
    ranks = np.asarray(ranks)

    t = x.shape[0]
    seg = np.searchsorted(b_start_loc, np.arange(t, dtype=b_start_loc.dtype),
                          side="right") - 1
    tok_adapter = b_adapter_ids[seg]
    tok_scale = b_scaling[seg]
    # this schedule assumes one adapter per 1024-token core slice (the
    # spec's equal-length-sequence layout guarantees it)
    assert all(
        len(np.unique(tok_adapter[c * TPC:(c + 1) * TPC])) == 1
        for c in range(N_CORES)
    )

    x16 = x.astype(np.float16)
    # pack W^T into [NQ*128, NT*2048]: row j*128+p, col n*2048 + q*512 + c
    # holds W^T[(4j+q)*128 + p, n*512 + c]
    wt = np.ascontiguousarray(weight.T.astype(np.float16))
    weight_p = np.ascontiguousarray(
        wt.reshape(NQ, QK, P, NT, NFREE)
          .transpose(0, 2, 3, 1, 4)
          .reshape(NQ * P, NT * QK * NFREE))

    in_maps = [
        _prep_core_inputs(x16, weight_p, bias, a_cache, b_cache, tok_adapter,
                          tok_scale, rank_page_table, ranks, c)
        for c in range(N_CORES)
    ]

    nc = _build_program()
    trace = os.environ.get("KERNEL_TRACE", "0") == "1"
    repeat = int(os.environ.get("KERNEL_REPEAT", "1"))
    times = []
    for _ in range(repeat):
        res = run_bass_kernel_spmd(nc, in_maps, core_ids=list(range(N_CORES)),
                                   trace=trace)
        times.append(res.exec_time_ns)
    last_exec_time_ns = (min(t for t in times if t is not None)
                         if any(t is not None for t in times) else None)
    last_results = res
    if repeat > 1:
        print("exec times:", times)
    return np.concatenate([res.results[c]["out"] for c in range(N_CORES)],
                          axis=0).astype(np.float32)


# revision 17
# speedup vs baseline: 1.0715x; 1.0020x over previous
"""LoRALinear (paged multi-adapter LoRA + base linear) Trainium2 kernel.

Full-input contract: kernel(**inputs) takes the unsharded tensors and
returns the full [T, D_OUT] output.

Sharding: tokens are split contiguously across the 8 NeuronCores
(1024 tokens/core).  The base weight, bias and the (tiny) LoRA page
caches are preprocessed on host into per-core dense operands:

  out_c = x_c @ W^T + bias + ((x_c @ A_c^T) * mask_c) @ B_c

All matmul operands are fp16 (rel err ~3e-4 vs the 2e-2 gate), which
runs the PE at the same 1 row/cycle as fp32r but halves weight-load
time and all DMA traffic.  PSUM accumulation stays fp32.  The bias is
folded into the LoRA-B matmul as one extra contraction row.

Device schedule (per core):
  - x^T (+ the 64 LoRA-A rows, fused into the same per-k slab DMA)
    stays resident in SBUF (~9 MiB fp16).
  - W is host-packed so each DMA is a [128, 2048] fp16 "quad" (4 KiB
    per partition row) holding 4 k-tiles of one n-block.
  - n=0's 8 quads stay resident, so the two m-tiles displaced by the
    LoRA-A PSUM accumulators during startup run right after the mask
    phase with zero extra DMA (the baseline re-streamed 8 MiB at the
    very end).
  - Output staging DMAs ride the scalar engine's queue and are split
    across DGE queues (the final tile's 256 KiB on one ~19 GB/s queue
    was a 12 us tail in the baseline).
"""

import os

import numpy as np

import concourse.bass as bass
import concourse.bacc as bacc
import concourse.mybir as mybir
import concourse.tile as tile
from concourse.bass_utils import run_bass_kernel_spmd

N_CORES = 8
T = 8192
D_IN = 4096
D_OUT = 4096
TPC = T // N_CORES  # tokens per core
MAX_RANK = 64
P = 128
NFREE = 512  # matmul moving free dim (psum bank)
KT = D_IN // P          # 32 k tiles
MT = TPC // P           # 8 token (m) tiles
NT = D_OUT // NFREE     # 8 output (n) blocks
QK = 4                  # k tiles per packed W quad
NQ = KT // QK           # 8 quads per n block
XAW = TPC + MAX_RANK    # fused x+A slab width (1088)

F32 = mybir.dt.float32
F16 = mybir.dt.float16

# exec time of the last device run (ns), when KERNEL_TRACE=1
last_exec_time_ns = None
last_results = None


def _build_program(r_aug=MAX_RANK + 1, d_out=D_OUT, tpc=TPC):
    """Build the per-core Bass program (G=1: 64 LoRA rows + bias row)."""
    t_chunks = tpc // NFREE  # 2
    m_inline = MT - t_chunks  # 6 m-tiles of n=0 run during startup
    defer = list(range(m_inline, MT))

    nc = bacc.Bacc("TRN2", target_bir_lowering=False, debug=False)

    xaT = nc.dram_tensor("xaT", [D_IN, XAW], F16, kind="ExternalInput").ap()
    wP = nc.dram_tensor("wP", [NQ * P, NT * QK * NFREE], F16,
                        kind="ExternalInput").ap()
    bS = nc.dram_tensor("bS", [r_aug, d_out], F16, kind="ExternalInput").ap()
    mS = nc.dram_tensor("mS", [r_aug, tpc], F32, kind="ExternalInput").ap()
    out = nc.dram_tensor("out", [tpc, d_out], F32, kind="ExternalOutput").ap()

    QW = QK * NFREE  # 2048 cols per quad

    with tile.TileContext(nc) as tc:
        with (
            tc.tile_pool(name="xapool", bufs=KT) as xapool,
            tc.tile_pool(name="w0pool", bufs=NQ) as w0pool,
            tc.tile_pool(name="cpool", bufs=1) as cpool,
            tc.tile_pool(name="wpool", bufs=6) as wpool,
            tc.tile_pool(name="opool", bufs=6) as opool,
            tc.tile_pool(name="psum", bufs=8, space="PSUM") as psum,
        ):
            # ---- DMA issue order: first k slab + first W quad lead ----
            # every big load is split along partitions so consecutive
            # dma_starts land on different DGE queues (~30 GB/s each)
            xs = []
            w0 = []

            def split_dma(eng, dst, src, parts):
                n_rows = dst.partition_size()
                rows = n_rows // parts
                for i in range(parts):
                    rs = slice(i * rows,
                               (i + 1) * rows if i < parts - 1 else n_rows)
                    eng.dma_start(dst[rs, :], src[rs, :])

            def xa_dma(k, parts=2):
                t = xapool.tile([P, XAW], F16, tag="xa", name=f"xa_{k}")
                split_dma(nc.sync, t, xaT[k * P:(k + 1) * P, :], parts)
                xs.append(t)

            def w0_dma(j, parts=2):
                t = w0pool.tile([P, QW], F16, tag="w0", name=f"w0_{j}")
                split_dma(nc.sync, t, wP[j * P:(j + 1) * P, 0:QW], parts)
                w0.append(t)

            xa_dma(0, parts=4)
            w0_dma(0, parts=4)
            xa_dma(1)
            w0_dma(1, parts=4)
            bss = cpool.tile([r_aug, d_out], F16, tag="bss", name="bss")
            mss = cpool.tile([r_aug, tpc], F32, tag="mss", name="mss")
            xam = cpool.tile([r_aug, tpc], F16, tag="xam", name="xam")
            for k in range(2, KT):
                xa_dma(k)
                j = k // QK + 1  # quad prefetch ~1 quad ahead of use
                if k % QK == 0 and j < NQ:
                    w0_dma(j)
                if k == 8:
                    # small mid-flight loads, only needed at mask time
                    split_dma(nc.sync, bss, bS, 2)
                    nc.sync.dma_start(mss, mS)

            # ones row for the bias contraction (mss row 64 is all 1.0)
            nc.vector.tensor_copy(xam[MAX_RANK:r_aug, :],
                                  mss[MAX_RANK:r_aug, :])

            # ---- PE startup: per k, LoRA-A + 6 inline n=0 m-tiles ----
            lora_ps = [psum.tile([MAX_RANK, NFREE], F32, tag="ps",
                                 name=f"ps_lora_{c}") for c in range(t_chunks)]
            psts0 = [psum.tile([P, NFREE], F32, tag="ps", name=f"pst0_{m}")
                     for m in range(m_inline)]
            for k in range(KT):
                wq = w0[k // QK]
                qs = (k % QK) * NFREE
                for c in range(t_chunks):
                    nc.tensor.matmul(
                        lora_ps[c],
                        lhsT=xs[k][:, tpc:tpc + MAX_RANK],
                        rhs=xs[k][:, c * NFREE:(c + 1) * NFREE],
                        start=(k == 0),
                        stop=(k == KT - 1),
                    )
                for m in range(m_inline):
                    nc.tensor.matmul(
                        psts0[m],
                        lhsT=xs[k][:, m * P:(m + 1) * P],
                        rhs=wq[:, qs:qs + NFREE],
                        start=(k == 0),
                        stop=False,
                    )

            # ---- masks: xam = lora_ps * mS (frees the two LoRA banks) ----
            for c in range(t_chunks):
                tsl = slice(c * NFREE, (c + 1) * NFREE)
                nc.vector.tensor_mul(xam[0:MAX_RANK, tsl],
                                     lora_ps[c],
                                     mss[0:MAX_RANK, tsl])

            def lora_b(pst, m, nsl):
                nc.tensor.matmul(
                    pst,
                    lhsT=xam[:, m * P:(m + 1) * P],
                    rhs=bss[:, nsl],
                    start=False,
                    stop=True,
                )

            def copy_out(m, n, pst, parts=2):
                ot = opool.tile([P, NFREE], F32, tag="ot", name=f"ot_{n}_{m}")
                nc.vector.tensor_copy(ot, pst)
                # scalar engine's DGE queue, split so rows land on several
                # queues (a single 256 KiB store drains at ~19 GB/s)
                rows = P // parts
                for i in range(parts):
                    nc.scalar.dma_start(
                        out[m * P + i * rows:m * P + (i + 1) * rows,
                            n * NFREE:(n + 1) * NFREE],
                        ot[i * rows:(i + 1) * rows, :])

            # ---- n=0 completion for the inline m-tiles ----
            n0sl = slice(0, NFREE)
            for m in range(m_inline):
                lora_b(psts0[m], m, n0sl)
                copy_out(m, 0, psts0[m])

            # ---- steady state: n = 1..7 ----
            for n in range(1, NT):
                nsl = slice(n * NFREE, (n + 1) * NFREE)
                psts = [psum.tile([P, NFREE], F32, tag="ps",
                                  name=f"pst_{n}_{m}") for m in range(MT)]
                for j in range(NQ):
                    wq = wpool.tile([P, QW], F16, tag="wt", name=f"wt_{n}_{j}")
                    split_dma(nc.sync, wq,
                              wP[j * P:(j + 1) * P, n * QW:(n + 1) * QW], 2)
                    for q in range(QK):
                        k = j * QK + q
                        for m in range(MT):
                            nc.tensor.matmul(
                                psts[m],
                                lhsT=xs[k][:, m * P:(m + 1) * P],
                                rhs=wq[:, q * NFREE:(q + 1) * NFREE],
                                start=(k == 0),
                                stop=False,
                            )
                            if k == KT - 1:
                                # finish tile m right away: the psum bank
                                # frees while the PE streams m+1..7, so the
                                # next n-block starts without a copy stall
                                lora_b(psts[m], m, nsl)
                                copy_out(m, n, psts[m])

            # ---- tail: the two n=0 m-tiles displaced by the LoRA-A psum
            # accumulators, straight from the resident quads (no DMA in,
            # only 512 KiB out -> short drain) ----
            pstd = [psum.tile([P, NFREE], F32, tag="ps", name=f"pstd_{m}")
                    for m in defer]
            for k in range(KT):
                wq = w0[k // QK]
                qs = (k % QK) * NFREE
                for i, m in enumerate(defer):
                    nc.tensor.matmul(
                        pstd[i],
                        lhsT=xs[k][:, m * P:(m + 1) * P],
                        rhs=wq[:, qs:qs + NFREE],
                        start=(k == 0),
                        stop=False,
                    )
                    if k == KT - 1:
                        lora_b(pstd[i], m, n0sl)
                        copy_out(m, 0, pstd[i])

    nc.compile()
    return nc


def _prep_core_inputs(x16, weight_p, bias, a_cache, b_cache, tok_adapter,
                      tok_scale, rank_page_table, ranks, core):
    """Host-side shard prep for one core (single adapter per core)."""
    d_out = b_cache.shape[1]
    r_aug = MAX_RANK + 1
    sl = slice(core * TPC, (core + 1) * TPC)
    adapters = tok_adapter[sl]
    scales = tok_scale[sl]
    a = int(adapters[0])
    pages = rank_page_table[a]  # [64] page ids

    xaT = np.empty((D_IN, XAW), np.float16)
    xaT[:, :TPC] = x16[sl].T
    xaT[:, TPC:] = a_cache[pages].T.astype(np.float16)

    bS = np.empty((r_aug, d_out), np.float16)
    bS[:MAX_RANK] = b_cache[pages].astype(np.float16)
    bS[MAX_RANK] = bias.astype(np.float16)

    mS = np.empty((r_aug, TPC), np.float32)
    slot_active = (np.arange(MAX_RANK) < ranks[a])[:, None]  # [64, 1]
    tok_active = (adapters == a)[None, :]  # [1, TPC]
    mS[:MAX_RANK] = (slot_active & tok_active) * scales[None, :]
    mS[MAX_RANK] = 1.0
    return {"xaT": xaT, "wP": weight_p, "bS": bS, "mS": mS}


def kernel(x, weight, bias, a_cache, b_cache, b_start_loc, b_adapter_ids,
           b_scaling, rank_page_table, ranks):
    global last_exec_time_ns, last_results
    x = np.asarray(x, np.float32)
    weight = np.asarray(weight, np.float32)
    bias = np.asarray(bias, np.float32)
    a_cache = np.asarray(a_cache, np.float32)
    b_cache = np.asarray(b_cache, np.float32)
    b_start_loc = np.asarray(b_start_loc)
    b_adapter_ids = np.asarray(b_adapter_ids)
    b_scaling = np.asarray(b_scaling, np.float32)
    rank_page_table = np.asarray(rank_page_table)
    ranks = np.asarray(ranks)

    t = x.shape[0]
    seg = np.searchsorted(b_start_loc, np.arange(t, dtype=b_start_loc.dtype),
                          side="right") - 1
    tok_adapter = b_adapter_ids[seg]
    tok_scale = b_scaling[seg]
    # this schedule assumes one adapter per 1024-token core slice (the
    # spec's equal-length-sequence layout guarantees it)
    assert all(
        len(np.unique(tok_adapter[c * TPC:(c + 1) * TPC])) == 1
        for c in range(N_CORES)
    )

    x16 = x.astype(np.float16)
    # pack W^T into [NQ*128, NT*2048]: row j*128+p, col n*2048 + q*512 + c
    # holds W^T[(4j+q)*128 + p, n*512 + c]
    wt = np.ascontiguousarray(weight.T.astype(np.float16))
    weight_p = np.ascontiguousarray(
        wt.reshape(NQ, QK, P, NT, NFREE)
          .transpose(0, 2, 3, 1, 4)
          .reshape(NQ * P, NT * QK * NFREE))

    in_maps = [
        _prep_core_inputs(x16, weight_p, bias, a_cache, b_cache, tok_adapter,
                          tok_scale, rank_page_table, ranks, c)
        for c in range(N_CORES)
    ]

    nc = _build_program()
    trace = os.environ.get("KERNEL_TRACE", "0") == "1"
    repeat = int(os.environ.get("KERNEL_REPEAT", "1"))
    times = []
    for _ in range(repeat):
        res = run_bass_kernel_spmd(nc, in_maps, core_ids=list(range(N_CORES)),
                                   trace=trace)
        times.append(res.exec_time_ns)
    last_exec_time_ns = (min(t for t in times if t is not None)
                         if any(t is not None for t in times) else None)
    last_results = res
    if repeat > 1:
        print("exec times:", times)
    return np.concatenate([res.results[c]["out"] for c in range(N_CORES)],
                          axis=0).astype(np.float32)


# revision 22
# speedup vs baseline: 1.0743x; 1.0027x over previous
"""LoRALinear (paged multi-adapter LoRA + base linear) Trainium2 kernel.

Full-input contract: kernel(**inputs) takes the unsharded tensors and
returns the full [T, D_OUT] output.

Sharding: tokens are split contiguously across the 8 NeuronCores
(1024 tokens/core).  The base weight, bias and the (tiny) LoRA page
caches are preprocessed on host into per-core dense operands:

  out_c = x_c @ W^T + bias + ((x_c @ A_c^T) * mask_c) @ B_c

All matmul operands are fp16 (rel err ~3e-4 vs the 2e-2 gate), which
runs the PE at the same 1 row/cycle as fp32r but halves weight-load
time and all DMA traffic.  PSUM accumulation stays fp32.  The bias is
folded into the LoRA-B matmul as one extra contraction row.

Device schedule (per core):
  - x^T (+ the 64 LoRA-A rows, fused into the same per-k slab DMA)
    stays resident in SBUF (~9 MiB fp16).
  - W is host-packed so each DMA is a [128, 2048] fp16 "quad" (4 KiB
    per partition row) holding 4 k-tiles of one n-block.
  - n=0's 8 quads stay resident, so the two m-tiles displaced by the
    LoRA-A PSUM accumulators during startup run right after the mask
    phase with zero extra DMA (the baseline re-streamed 8 MiB at the
    very end).
  - Output staging DMAs ride the scalar engine's queue and are split
    across DGE queues (the final tile's 256 KiB on one ~19 GB/s queue
    was a 12 us tail in the baseline).
"""

import os

import numpy as np

import concourse.bass as bass
import concourse.bacc as bacc
import concourse.mybir as mybir
import concourse.tile as tile
from concourse.bass_utils import run_bass_kernel_spmd

N_CORES = 8
T = 8192
D_IN = 4096
D_OUT = 4096
TPC = T // N_CORES  # tokens per core
MAX_RANK = 64
P = 128
NFREE = 512  # matmul moving free dim (psum bank)
KT = D_IN // P          # 32 k tiles
MT = TPC // P           # 8 token (m) tiles
NT = D_OUT // NFREE     # 8 output (n) blocks
QK = 4                  # k tiles per packed W quad
NQ = KT // QK           # 8 quads per n block
XAW = TPC + MAX_RANK    # fused x+A slab width (1088)

F32 = mybir.dt.float32
F16 = mybir.dt.float16

# exec time of the last device run (ns), when KERNEL_TRACE=1
last_exec_time_ns = None
last_results = None


def _build_program(r_aug=MAX_RANK + 1, d_out=D_OUT, tpc=TPC):
    """Build the per-core Bass program (G=1: 64 LoRA rows + bias row)."""
    t_chunks = tpc // NFREE  # 2
    m_inline = MT - t_chunks  # 6 m-tiles of n=0 run during startup
    defer = list(range(m_inline, MT))

    nc = bacc.Bacc("TRN2", target_bir_lowering=False, debug=False)

    xaT = nc.dram_tensor("xaT", [D_IN, XAW], F16, kind="ExternalInput").ap()
    wP = nc.dram_tensor("wP", [NQ * P, NT * QK * NFREE], F16,
                        kind="ExternalInput").ap()
    bS = nc.dram_tensor("bS", [r_aug, d_out], F16, kind="ExternalInput").ap()
    mS = nc.dram_tensor("mS", [r_aug, tpc], F32, kind="ExternalInput").ap()
    out = nc.dram_tensor("out", [tpc, d_out], F32, kind="ExternalOutput").ap()

    QW = QK * NFREE  # 2048 cols per quad

    with tile.TileContext(nc) as tc:
        with (
            tc.tile_pool(name="xapool", bufs=KT) as xapool,
            tc.tile_pool(name="w0pool", bufs=NQ) as w0pool,
            tc.tile_pool(name="cpool", bufs=1) as cpool,
            tc.tile_pool(name="wpool", bufs=6) as wpool,
            tc.tile_pool(name="opool", bufs=6) as opool,
            tc.tile_pool(name="psum", bufs=8, space="PSUM") as psum,
        ):
            # ---- DMA issue order: first k slab + first W quad lead ----
            # every big load is split along partitions so consecutive
            # dma_starts land on different DGE queues (~30 GB/s each)
            xs = []
            w0 = []

            def split_dma(eng, dst, src, parts):
                n_rows = dst.partition_size()
                rows = n_rows // parts
                for i in range(parts):
                    rs = slice(i * rows,
                               (i + 1) * rows if i < parts - 1 else n_rows)
                    eng.dma_start(dst[rs, :], src[rs, :])

            def xa_dma(k, parts=2):
                t = xapool.tile([P, XAW], F16, tag="xa", name=f"xa_{k}")
                split_dma(nc.sync, t, xaT[k * P:(k + 1) * P, :], parts)
                xs.append(t)

            def w0_dma(j, parts=2):
                t = w0pool.tile([P, QW], F16, tag="w0", name=f"w0_{j}")
                split_dma(nc.sync, t, wP[j * P:(j + 1) * P, 0:QW], parts)
                w0.append(t)

            xa_dma(0, parts=4)
            w0_dma(0, parts=4)
            xa_dma(1)
            w0_dma(1, parts=4)
            bss = cpool.tile([r_aug, d_out], F16, tag="bss", name="bss")
            mss = cpool.tile([r_aug, tpc], F32, tag="mss", name="mss")
            xam = cpool.tile([r_aug, tpc], F16, tag="xam", name="xam")
            dummy = cpool.tile([P, NFREE], F16, tag="dummy", name="dummy")
            for k in range(2, KT):
                xa_dma(k)
                j = k // QK + 1  # quad prefetch ~1 quad ahead of use
                if k % QK == 0 and j < NQ:
                    w0_dma(j)
                if k == 20:
                    # small mid-flight loads, only needed at mask time
                    # (issued late so they don't delay the startup slabs)
                    split_dma(nc.sync, bss, bS, 2)
                    split_dma(nc.sync, mss, mS, 2)

            # warm-up: garbage matmuls with no input DMA dependency keep
            # the PE busy while the first slab lands and finish the ~3 us
            # p-state ramp before real work arrives (shares the psum slot
            # rotation, so a later tile's first write waits for them — they
            # are long done by then)
            nc.vector.memset(dummy, 0.0)
            warm_ps = psum.tile([P, NFREE], F32, tag="ps", name="warm_ps")
            for i in range(12):
                nc.tensor.matmul(warm_ps, lhsT=dummy[:, 0:P],
                                 rhs=dummy, start=True, stop=True)

            # ones row for the bias contraction (mss row 64 is all 1.0)
            nc.vector.tensor_copy(xam[MAX_RANK:r_aug, :],
                                  mss[MAX_RANK:r_aug, :])

            # ---- PE startup: per k, LoRA-A + 6 inline n=0 m-tiles ----
            lora_ps = [psum.tile([MAX_RANK, NFREE], F32, tag="ps",
                                 name=f"ps_lora_{c}") for c in range(t_chunks)]
            psts0 = [psum.tile([P, NFREE], F32, tag="ps", name=f"pst0_{m}")
                     for m in range(m_inline)]
            for k in range(KT):
                wq = w0[k // QK]
                qs = (k % QK) * NFREE
                for c in range(t_chunks):
                    nc.tensor.matmul(
                        lora_ps[c],
                        lhsT=xs[k][:, tpc:tpc + MAX_RANK],
                        rhs=xs[k][:, c * NFREE:(c + 1) * NFREE],
                        start=(k == 0),
                        stop=(k == KT - 1),
                    )
                for m in range(m_inline):
                    nc.tensor.matmul(
                        psts0[m],
                        lhsT=xs[k][:, m * P:(m + 1) * P],
                        rhs=wq[:, qs:qs + NFREE],
                        start=(k == 0),
                        stop=False,
                    )

            # ---- masks: xam = lora_ps * mS (frees the two LoRA banks) ----
            for c in range(t_chunks):
                tsl = slice(c * NFREE, (c + 1) * NFREE)
                nc.vector.tensor_mul(xam[0:MAX_RANK, tsl],
                                     lora_ps[c],
                                     mss[0:MAX_RANK, tsl])

            def lora_b(pst, m, nsl):
                nc.tensor.matmul(
                    pst,
                    lhsT=xam[:, m * P:(m + 1) * P],
                    rhs=bss[:, nsl],
                    start=False,
                    stop=True,
                )

            def copy_out(m, n, pst, parts=2, eng=None):
                ot = opool.tile([P, NFREE], F32, tag="ot", name=f"ot_{n}_{m}")
                nc.vector.tensor_copy(ot, pst)
                # scalar engine's DGE queue, split so rows land on several
                # queues (a single 256 KiB store drains at ~19 GB/s)
                rows = P // parts
                for i in range(parts):
                    (eng or nc.scalar).dma_start(
                        out[m * P + i * rows:m * P + (i + 1) * rows,
                            n * NFREE:(n + 1) * NFREE],
                        ot[i * rows:(i + 1) * rows, :])

            # ---- n=0 completion for the inline m-tiles ----
            n0sl = slice(0, NFREE)
            for m in range(m_inline):
                lora_b(psts0[m], m, n0sl)
                copy_out(m, 0, psts0[m])

            # ---- steady state: n = 1..7 ----
            for n in range(1, NT):
                nsl = slice(n * NFREE, (n + 1) * NFREE)
                psts = [psum.tile([P, NFREE], F32, tag="ps",
                                  name=f"pst_{n}_{m}") for m in range(MT)]
                for j in range(NQ):
                    wq = wpool.tile([P, QW], F16, tag="wt", name=f"wt_{n}_{j}")
                    split_dma(nc.sync, wq,
                              wP[j * P:(j + 1) * P, n * QW:(n + 1) * QW], 2)
                    for q in range(QK):
                        k = j * QK + q
                        for m in range(MT):
                            nc.tensor.matmul(
                                psts[m],
                                lhsT=xs[k][:, m * P:(m + 1) * P],
                                rhs=wq[:, q * NFREE:(q + 1) * NFREE],
                                start=(k == 0),
                                stop=False,
                            )
                            if k == KT - 1:
                                # finish tile m right away: the psum bank
                                # frees while the PE streams m+1..7, so the
                                # next n-block starts without a copy stall
                                lora_b(psts[m], m, nsl)
                                copy_out(m, n, psts[m])

            # ---- tail: the two n=0 m-tiles displaced by the LoRA-A psum
            # accumulators, straight from the resident quads (no DMA in,
            # only 512 KiB out -> short drain) ----
            pstd = [psum.tile([P, NFREE], F32, tag="ps", name=f"pstd_{m}")
                    for m in defer]
            for k in range(KT):
                wq = w0[k // QK]
                qs = (k % QK) * NFREE
                for i, m in enumerate(defer):
                    nc.tensor.matmul(
                        pstd[i],
                        lhsT=xs[k][:, m * P:(m + 1) * P],
                        rhs=wq[:, qs:qs + NFREE],
                        start=(k == 0),
                        stop=False,
                    )
                    if k == KT - 1:
                        lora_b(pstd[i], m, n0sl)
                        # final stores: 4-way splits, issue queues spread
                        # over two engines so the drain parallelizes
                        copy_out(m, 0, pstd[i], parts=4,
                                 eng=nc.sync if i == 0 else nc.scalar)

    nc.compile()
    return nc


def _prep_core_inputs(x16, weight_p, bias, a_cache, b_cache, tok_adapter,
                      tok_scale, rank_page_table, ranks, core):
    """Host-side shard prep for one core (single adapter per core)."""
    d_out = b_cache.shape[1]
    r_aug = MAX_RANK + 1
    sl = slice(core * TPC, (core + 1) * TPC)
    adapters = tok_adapter[sl]
    scales = tok_scale[sl]
    a = int(adapters[0])
    pages = rank_page_table[a]  # [64] page ids

    xaT = np.empty((D_IN, XAW), np.float16)
    xaT[:, :TPC] = x16[sl].T
    xaT[:, TPC:] = a_cache[pages].T.astype(np.float16)

    bS = np.empty((r_aug, d_out), np.float16)
    bS[:MAX_RANK] = b_cache[pages].astype(np.float16)
    bS[MAX_RANK] = bias.astype(np.float16)

    mS = np.empty((r_aug, TPC), np.float32)
    slot_active = (np.arange(MAX_RANK) < ranks[a])[:, None]  # [64, 1]
    tok_active = (adapters == a)[None, :]  # [1, TPC]
    mS[:MAX_RANK] = (slot_active & tok_active) * scales[None, :]
    mS[MAX_RANK] = 1.0
    return {"xaT": xaT, "wP": weight_p, "bS": bS, "mS": mS}


def kernel(x, weight, bias, a_cache, b_cache, b_start_loc, b_adapter_ids,
           b_scaling, rank_page_table, ranks):
    global last_exec_time_ns, last_results
    x = np.asarray(x, np.float32)
    weight = np.asarray(weight, np.float32)
    bias = np.asarray(bias, np.float32)
    a_cache = np.asarray(a_cache, np.float32)
    b_cache = np.asarray(b_cache, np.float32)
    b_start_loc = np.asarray(b_start_loc)
    b_adapter_ids = np.asarray(b_adapter_ids)
    b_scaling = np.asarray(b_scaling, np.float32)
    rank_page_table = np.asarray(rank_page_table)
    ranks = np.asarray(ranks)

    t = x.shape[0]
    seg = np.searchsorted(b_start_loc, np.arange(t, dtype=b_start_loc.dtype),
                          side="right") - 1
    tok_adapter = b_adapter_ids[seg]
    tok_scale = b_scaling[seg]
    # this schedule assumes one adapter per 1024-token core slice (the
    # spec's equal-length-sequence layout guarantees it)
    assert all(
        len(np.unique(tok_adapter[c * TPC:(c + 1) * TPC])) == 1
        for c in range(N_CORES)
    )

    x16 = x.astype(np.float16)
    # pack W^T into [NQ*128, NT*2048]: row j*128+p, col n*2048 + q*512 + c
    # holds W^T[(4j+q)*128 + p, n*512 + c]
    wt = np.ascontiguousarray(weight.T.astype(np.float16))
    weight_p = np.ascontiguousarray(
        wt.reshape(NQ, QK, P, NT, NFREE)
          .transpose(0, 2, 3, 1, 4)
          .reshape(NQ * P, NT * QK * NFREE))

    in_maps = [
        _prep_core_inputs(x16, weight_p, bias, a_cache, b_cache, tok_adapter,
                          tok_scale, rank_page_table, ranks, c)
        for c in range(N_CORES)
    ]

    nc = _build_program()
    trace = os.environ.get("KERNEL_TRACE", "0") == "1"
    repeat = int(os.environ.get("KERNEL_REPEAT", "1"))
    times = []
    for _ in range(repeat):
        res = run_bass_kernel_spmd(nc, in_maps, core_ids=list(range(N_CORES)),
                                   trace=trace)
        times.append(res.exec_time_ns)
    last_exec_time_ns = (min(t for t in times if t is not None)
                         if any(t is not None for t in times) else None)
    last_results = res
    if repeat > 1:
        print("exec times:", times)
    return np.concatenate([res.results[c]["out"] for c in range(N_CORES)],
                          axis=0).astype(np.float32)


# revision 24
# speedup vs baseline: 1.0774x; 1.0028x over previous
"""LoRALinear (paged multi-adapter LoRA + base linear) Trainium2 kernel.

Full-input contract: kernel(**inputs) takes the unsharded tensors and
returns the full [T, D_OUT] output.

Sharding: tokens are split contiguously across the 8 NeuronCores
(1024 tokens/core).  The base weight, bias and the (tiny) LoRA page
caches are preprocessed on host into per-core dense operands:

  out_c = x_c @ W^T + bias + ((x_c @ A_c^T) * mask_c) @ B_c

All matmul operands are fp16 (rel err ~3e-4 vs the 2e-2 gate), which
runs the PE at the same 1 row/cycle as fp32r but halves weight-load
time and all DMA traffic.  PSUM accumulation stays fp32.  The bias is
folded into the LoRA-B matmul as one extra contraction row.

Device schedule (per core):
  - x^T (+ the 64 LoRA-A rows, fused into the same per-k slab DMA)
    stays resident in SBUF (~9 MiB fp16).
  - W is host-packed so each DMA is a [128, 2048] fp16 "quad" (4 KiB
    per partition row) holding 4 k-tiles of one n-block.
  - n=0's 8 quads stay resident, so the two m-tiles displaced by the
    LoRA-A PSUM accumulators during startup run right after the mask
    phase with zero extra DMA (the baseline re-streamed 8 MiB at the
    very end).
  - Output staging DMAs ride the scalar engine's queue and are split
    across DGE queues (the final tile's 256 KiB on one ~19 GB/s queue
    was a 12 us tail in the baseline).
"""

import os

import numpy as np

import concourse.bass as bass
import concourse.bacc as bacc
import concourse.mybir as mybir
import concourse.tile as tile
from concourse.bass_utils import run_bass_kernel_spmd

N_CORES = 8
T = 8192
D_IN = 4096
D_OUT = 4096
TPC = T // N_CORES  # tokens per core
MAX_RANK = 64
P = 128
NFREE = 512  # matmul moving free dim (psum bank)
KT = D_IN // P          # 32 k tiles
MT = TPC // P           # 8 token (m) tiles
NT = D_OUT // NFREE     # 8 output (n) blocks
QK = 4                  # k tiles per packed W quad
NQ = KT // QK           # 8 quads per n block
XAW = TPC + MAX_RANK    # fused x+A slab width (1088)

F32 = mybir.dt.float32
F16 = mybir.dt.float16

# exec time of the last device run (ns), when KERNEL_TRACE=1
last_exec_time_ns = None
last_results = None


def _build_program(r_aug=MAX_RANK + 1, d_out=D_OUT, tpc=TPC):
    """Build the per-core Bass program (G=1: 64 LoRA rows + bias row)."""
    t_chunks = tpc // NFREE  # 2
    m_inline = MT - t_chunks  # 6 m-tiles of n=0 run during startup
    defer = list(range(m_inline, MT))

    nc = bacc.Bacc("TRN2", target_bir_lowering=False, debug=False)

    xaT = nc.dram_tensor("xaT", [D_IN, XAW], F16, kind="ExternalInput").ap()
    wP = nc.dram_tensor("wP", [NQ * P, NT * QK * NFREE], F16,
                        kind="ExternalInput").ap()
    bS = nc.dram_tensor("bS", [r_aug, d_out], F16, kind="ExternalInput").ap()
    mS = nc.dram_tensor("mS", [r_aug, tpc], F32, kind="ExternalInput").ap()
    out = nc.dram_tensor("out", [tpc, d_out], F32, kind="ExternalOutput").ap()

    QW = QK * NFREE  # 2048 cols per quad

    with tile.TileContext(nc) as tc:
        with (
            tc.tile_pool(name="xapool", bufs=KT) as xapool,
            tc.tile_pool(name="w0pool", bufs=NQ) as w0pool,
            tc.tile_pool(name="cpool", bufs=1) as cpool,
            tc.tile_pool(name="wpool", bufs=8) as wpool,
            tc.tile_pool(name="opool", bufs=6) as opool,
            tc.tile_pool(name="psum", bufs=8, space="PSUM") as psum,
        ):
            # ---- DMA issue order: first k slab + first W quad lead ----
            # every big load is split along partitions so consecutive
            # dma_starts land on different DGE queues (~30 GB/s each)
            xs = []
            w0 = []

            def split_dma(eng, dst, src, parts):
                n_rows = dst.partition_size()
                rows = n_rows // parts
                for i in range(parts):
                    rs = slice(i * rows,
                               (i + 1) * rows if i < parts - 1 else n_rows)
                    eng.dma_start(dst[rs, :], src[rs, :])

            def dual_dma(dst, src, parts):
                # issue alternately from sync and scalar so the issue
                # serialization (~0.6 us per dma_start) halves
                n_rows = dst.partition_size()
                rows = n_rows // parts
                for i in range(parts):
                    rs = slice(i * rows,
                               (i + 1) * rows if i < parts - 1 else n_rows)
                    (nc.sync if i % 2 == 0 else nc.scalar).dma_start(
                        dst[rs, :], src[rs, :])

            def xa_dma(k, parts=2):
                t = xapool.tile([P, XAW], F16, tag="xa", name=f"xa_{k}")
                split_dma(nc.sync, t, xaT[k * P:(k + 1) * P, :], parts)
                xs.append(t)

            def w0_dma(j, parts=2):
                t = w0pool.tile([P, QW], F16, tag="w0", name=f"w0_{j}")
                split_dma(nc.sync, t, wP[j * P:(j + 1) * P, 0:QW], parts)
                w0.append(t)

            # first loads: issue from both queues in parallel
            xa0 = xapool.tile([P, XAW], F16, tag="xa", name="xa_0")
            dual_dma(xa0, xaT[0:P, :], 4)
            xs.append(xa0)
            w00 = w0pool.tile([P, QW], F16, tag="w0", name="w0_0")
            dual_dma(w00, wP[0:P, 0:QW], 4)
            w0.append(w00)
            xa1 = xapool.tile([P, XAW], F16, tag="xa", name="xa_1")
            dual_dma(xa1, xaT[P:2 * P, :], 2)
            xs.append(xa1)
            w01 = w0pool.tile([P, QW], F16, tag="w0", name="w0_1")
            dual_dma(w01, wP[P:2 * P, 0:QW], 4)
            w0.append(w01)
            bss = cpool.tile([r_aug, d_out], F16, tag="bss", name="bss")
            mss = cpool.tile([r_aug, tpc], F32, tag="mss", name="mss")
            xam = cpool.tile([r_aug, tpc], F16, tag="xam", name="xam")
            dummy = cpool.tile([P, NFREE], F16, tag="dummy", name="dummy")
            for k in range(2, KT):
                xa_dma(k)
                j = k // QK + 1  # quad prefetch ~1 quad ahead of use
                if k % QK == 0 and j < NQ:
                    w0_dma(j)
                if k == 20:
                    # small mid-flight loads, only needed at mask time
                    # (issued late so they don't delay the startup slabs)
                    split_dma(nc.sync, bss, bS, 2)
                    split_dma(nc.sync, mss, mS, 2)

            # warm-up: garbage matmuls with no input DMA dependency keep
            # the PE busy while the first slab lands and finish the ~3 us
            # p-state ramp before real work arrives (shares the psum slot
            # rotation, so a later tile's first write waits for them — they
            # are long done by then)
            nc.vector.memset(dummy, 0.0)
            warm_ps = psum.tile([P, NFREE], F32, tag="ps", name="warm_ps")
            for i in range(12):
                nc.tensor.matmul(warm_ps, lhsT=dummy[:, 0:P],
                                 rhs=dummy, start=True, stop=True)

            # ones row for the bias contraction (mss row 64 is all 1.0)
            nc.vector.tensor_copy(xam[MAX_RANK:r_aug, :],
                                  mss[MAX_RANK:r_aug, :])

            # ---- PE startup: per k, LoRA-A + 6 inline n=0 m-tiles ----
            lora_ps = [psum.tile([MAX_RANK, NFREE], F32, tag="ps",
                                 name=f"ps_lora_{c}") for c in range(t_chunks)]
            psts0 = [psum.tile([P, NFREE], F32, tag="ps", name=f"pst0_{m}")
                     for m in range(m_inline)]
            for k in range(KT):
                wq = w0[k // QK]
                qs = (k % QK) * NFREE
                for c in range(t_chunks):
                    nc.tensor.matmul(
                        lora_ps[c],
                        lhsT=xs[k][:, tpc:tpc + MAX_RANK],
                        rhs=xs[k][:, c * NFREE:(c + 1) * NFREE],
                        start=(k == 0),
                        stop=(k == KT - 1),
                    )
                for m in range(m_inline):
                    nc.tensor.matmul(
                        psts0[m],
                        lhsT=xs[k][:, m * P:(m + 1) * P],
                        rhs=wq[:, qs:qs + NFREE],
                        start=(k == 0),
                        stop=False,
                    )

            # ---- masks: xam = lora_ps * mS (frees the two LoRA banks) ----
            for c in range(t_chunks):
                tsl = slice(c * NFREE, (c + 1) * NFREE)
                nc.vector.tensor_mul(xam[0:MAX_RANK, tsl],
                                     lora_ps[c],
                                     mss[0:MAX_RANK, tsl])

            def lora_b(pst, m, nsl):
                nc.tensor.matmul(
                    pst,
                    lhsT=xam[:, m * P:(m + 1) * P],
                    rhs=bss[:, nsl],
                    start=False,
                    stop=True,
                )

            def copy_out(m, n, pst, parts=2, eng=None):
                ot = opool.tile([P, NFREE], F32, tag="ot", name=f"ot_{n}_{m}")
                nc.vector.tensor_copy(ot, pst)
                # scalar engine's DGE queue, split so rows land on several
                # queues (a single 256 KiB store drains at ~19 GB/s)
                rows = P // parts
                for i in range(parts):
                    (eng or nc.scalar).dma_start(
                        out[m * P + i * rows:m * P + (i + 1) * rows,
                            n * NFREE:(n + 1) * NFREE],
                        ot[i * rows:(i + 1) * rows, :])

            # ---- n=0 completion for the inline m-tiles ----
            n0sl = slice(0, NFREE)
            for m in range(m_inline):
                lora_b(psts0[m], m, n0sl)
                copy_out(m, 0, psts0[m])

            # ---- steady state: n = 1..7 ----
            for n in range(1, NT):
                nsl = slice(n * NFREE, (n + 1) * NFREE)
                psts = [psum.tile([P, NFREE], F32, tag="ps",
                                  name=f"pst_{n}_{m}") for m in range(MT)]
                for j in range(NQ):
                    wq = wpool.tile([P, QW], F16, tag="wt", name=f"wt_{n}_{j}")
                    split_dma(nc.sync, wq,
                              wP[j * P:(j + 1) * P, n * QW:(n + 1) * QW], 2)
                    for q in range(QK):
                        k = j * QK + q
                        for m in range(MT):
                            nc.tensor.matmul(
                                psts[m],
                                lhsT=xs[k][:, m * P:(m + 1) * P],
                                rhs=wq[:, q * NFREE:(q + 1) * NFREE],
                                start=(k == 0),
                                stop=False,
                            )
                            if k == KT - 1:
                                # finish tile m right away: the psum bank
                                # frees while the PE streams m+1..7, so the
                                # next n-block starts without a copy stall
                                lora_b(psts[m], m, nsl)
                                copy_out(m, n, psts[m])

            # ---- tail: the two n=0 m-tiles displaced by the LoRA-A psum
            # accumulators, straight from the resident quads (no DMA in,
            # only 512 KiB out -> short drain) ----
            pstd = [psum.tile([P, NFREE], F32, tag="ps", name=f"pstd_{m}")
                    for m in defer]
            for k in range(KT):
                wq = w0[k // QK]
                qs = (k % QK) * NFREE
                for i, m in enumerate(defer):
                    nc.tensor.matmul(
                        pstd[i],
                        lhsT=xs[k][:, m * P:(m + 1) * P],
                        rhs=wq[:, qs:qs + NFREE],
                        start=(k == 0),
                        stop=False,
                    )
                    if k == KT - 1:
                        lora_b(pstd[i], m, n0sl)
                        # final stores: 4-way splits, issue queues spread
                        # over two engines so the drain parallelizes
                        copy_out(m, 0, pstd[i], parts=4,
                                 eng=nc.sync if i == 0 else nc.scalar)

    nc.compile()
    return nc


def _prep_core_inputs(x16, weight_p, bias, a_cache, b_cache, tok_adapter,
                      tok_scale, rank_page_table, ranks, core):
    """Host-side shard prep for one core (single adapter per core)."""
    d_out = b_cache.shape[1]
    r_aug = MAX_RANK + 1
    sl = slice(core * TPC, (core + 1) * TPC)
    adapters = tok_adapter[sl]
    scales = tok_scale[sl]
    a = int(adapters[0])
    pages = rank_page_table[a]  # [64] page ids

    xaT = np.empty((D_IN, XAW), np.float16)
    xaT[:, :TPC] = x16[sl].T
    xaT[:, TPC:] = a_cache[pages].T.astype(np.float16)

    bS = np.empty((r_aug, d_out), np.float16)
    bS[:MAX_RANK] = b_cache[pages].astype(np.float16)
    bS[MAX_RANK] = bias.astype(np.float16)

    mS = np.empty((r_aug, TPC), np.float32)
    slot_active = (np.arange(MAX_RANK) < ranks[a])[:, None]  # [64, 1]
    tok_active = (adapters == a)[None, :]  # [1, TPC]
    mS[:MAX_RANK] = (slot_active & tok_active) * scales[None, :]
    mS[MAX_RANK] = 1.0
    return {"xaT": xaT, "wP": weight_p, "bS": bS, "mS": mS}


def kernel(x, weight, bias, a_cache, b_cache, b_start_loc, b_adapter_ids,
           b_scaling, rank_page_table, ranks):
    global last_exec_time_ns, last_results
    x = np.asarray(x, np.float32)
    weight = np.asarray(weight, np.float32)
    bias = np.asarray(bias, np.float32)
    a_cache = np.asarray(a_cache, np.float32)
    b_cache = np.asarray(b_cache, np.float32)
    b_start_loc = np.asarray(b_start_loc)
    b_adapter_ids = np.asarray(b_adapter_ids)
    b_scaling = np.asarray(b_scaling, np.float32)
    rank_page_table = np.asarray(rank_page_table)
    ranks = np.asarray(ranks)

    t = x.shape[0]
    seg = np.searchsorted(b_start_loc, np.arange(t, dtype=b_start_loc.dtype),
                          side="right") - 1
    tok_adapter = b_adapter_ids[seg]
    tok_scale = b_scaling[seg]
    # this schedule assumes one adapter per 1024-token core slice (the
    # spec's equal-length-sequence layout guarantees it)
    assert all(
        len(np.unique(tok_adapter[c * TPC:(c + 1) * TPC])) == 1
        for c in range(N_CORES)
    )

    x16 = x.astype(np.float16)
    # pack W^T into [NQ*128, NT*2048]: row j*128+p, col n*2048 + q*512 + c
    # holds W^T[(4j+q)*128 + p, n*512 + c]
    wt = np.ascontiguousarray(weight.T.astype(np.float16))
    weight_p = np.ascontiguousarray(
        wt.reshape(NQ, QK, P, NT, NFREE)
          .transpose(0, 2, 3, 1, 4)
          .reshape(NQ * P, NT * QK * NFREE))

    in_maps = [
        _prep_core_inputs(x16, weight_p, bias, a_cache, b_cache, tok_adapter,
                          tok_scale, rank_page_table, ranks, c)
        for c in range(N_CORES)
    ]

    nc = _build_program()
    trace = os.environ.get("KERNEL_TRACE", "0") == "1"
    repeat = int(os.environ.get("KERNEL_REPEAT", "1"))
    times = []
    for _ in range(repeat):
        res = run_bass_kernel_spmd(nc, in_maps, core_ids=list(range(N_CORES)),
                                   trace=trace)
        times.append(res.exec_time_ns)
    last_exec_time_ns = (min(t for t in times if t is not None)
                         if any(t is not None for t in times) else None)
    last_results = res
    if repeat > 1:
        print("exec times:", times)
    return np.concatenate([res.results[c]["out"] for c in range(N_CORES)],
                          axis=0).astype(np.float32)


# revision 25
# speedup vs baseline: 1.0961x; 1.0174x over previous
"""LoRALinear (paged multi-adapter LoRA + base linear) Trainium2 kernel.

Full-input contract: kernel(**inputs) takes the unsharded tensors and
returns the full [T, D_OUT] output.

Sharding: tokens are split contiguously across the 8 NeuronCores
(1024 tokens/core).  The base weight, bias and the (tiny) LoRA page
caches are preprocessed on host into per-core dense operands:

  out_c = x_c @ W^T + bias + ((x_c @ A_c^T) * mask_c) @ B_c

All matmul operands are fp16 (rel err ~3e-4 vs the 2e-2 gate), which
runs the PE at the same 1 row/cycle as fp32r but halves weight-load
time and all DMA traffic.  PSUM accumulation stays fp32.  The bias is
folded into the LoRA-B matmul as one extra contraction row.

Device schedule (per core):
  - x^T (+ the 64 LoRA-A rows, fused into the same per-k slab DMA)
    stays resident in SBUF (~9 MiB fp16).
  - W is host-packed so each DMA is a [128, 2048] fp16 "quad" (4 KiB
    per partition row) holding 4 k-tiles of one n-block.
  - n=0's 8 quads stay resident, so the two m-tiles displaced by the
    LoRA-A PSUM accumulators during startup run right after the mask
    phase with zero extra DMA (the baseline re-streamed 8 MiB at the
    very end).
  - Output staging DMAs ride the scalar engine's queue and are split
    across DGE queues (the final tile's 256 KiB on one ~19 GB/s queue
    was a 12 us tail in the baseline).
"""

import os

import numpy as np

import concourse.bass as bass
import concourse.bacc as bacc
import concourse.mybir as mybir
import concourse.tile as tile
from concourse.bass_utils import run_bass_kernel_spmd

N_CORES = 8
T = 8192
D_IN = 4096
D_OUT = 4096
TPC = T // N_CORES  # tokens per core
MAX_RANK = 64
P = 128
NFREE = 512  # matmul moving free dim (psum bank)
KT = D_IN // P          # 32 k tiles
MT = TPC // P           # 8 token (m) tiles
NT = D_OUT // NFREE     # 8 output (n) blocks
QK = 4                  # k tiles per packed W quad
NQ = KT // QK           # 8 quads per n block
XAW = TPC + MAX_RANK    # fused x+A slab width (1088)

F32 = mybir.dt.float32
F16 = mybir.dt.float16

# exec time of the last device run (ns), when KERNEL_TRACE=1
last_exec_time_ns = None
last_results = None


def _build_program(r_aug=MAX_RANK + 1, d_out=D_OUT, tpc=TPC):
    """Build the per-core Bass program (G=1: 64 LoRA rows + bias row)."""
    t_chunks = tpc // NFREE  # 2
    # both LoRA-A accumulators pack into ONE psum bank ([64,512] at
    # partitions 0:64 and 64:128), so 7 m-tiles of n=0 run inline
    m_inline = MT - 1
    defer = list(range(m_inline, MT))

    nc = bacc.Bacc("TRN2", target_bir_lowering=False, debug=False)

    xaT = nc.dram_tensor("xaT", [D_IN, XAW], F16, kind="ExternalInput").ap()
    wP = nc.dram_tensor("wP", [NQ * P, NT * QK * NFREE], F16,
                        kind="ExternalInput").ap()
    bS = nc.dram_tensor("bS", [r_aug, d_out], F16, kind="ExternalInput").ap()
    mS = nc.dram_tensor("mS", [r_aug, tpc], F32, kind="ExternalInput").ap()
    out = nc.dram_tensor("out", [tpc, d_out], F32, kind="ExternalOutput").ap()

    QW = QK * NFREE  # 2048 cols per quad

    with tile.TileContext(nc) as tc:
        with (
            tc.tile_pool(name="xapool", bufs=KT) as xapool,
            tc.tile_pool(name="w0pool", bufs=NQ) as w0pool,
            tc.tile_pool(name="cpool", bufs=1) as cpool,
            tc.tile_pool(name="wpool", bufs=8) as wpool,
            tc.tile_pool(name="opool", bufs=6) as opool,
            tc.tile_pool(name="psum", bufs=8, space="PSUM") as psum,
        ):
            # ---- DMA issue order: first k slab + first W quad lead ----
            # every big load is split along partitions so consecutive
            # dma_starts land on different DGE queues (~30 GB/s each)
            xs = []
            w0 = []

            def split_dma(eng, dst, src, parts):
                n_rows = dst.partition_size()
                rows = n_rows // parts
                for i in range(parts):
                    rs = slice(i * rows,
                               (i + 1) * rows if i < parts - 1 else n_rows)
                    eng.dma_start(dst[rs, :], src[rs, :])

            def dual_dma(dst, src, parts):
                # issue alternately from sync and scalar so the issue
                # serialization (~0.6 us per dma_start) halves
                n_rows = dst.partition_size()
                rows = n_rows // parts
                for i in range(parts):
                    rs = slice(i * rows,
                               (i + 1) * rows if i < parts - 1 else n_rows)
                    (nc.sync if i % 2 == 0 else nc.scalar).dma_start(
                        dst[rs, :], src[rs, :])

            def xa_dma(k, parts=2):
                t = xapool.tile([P, XAW], F16, tag="xa", name=f"xa_{k}")
                split_dma(nc.sync, t, xaT[k * P:(k + 1) * P, :], parts)
                xs.append(t)

            def w0_dma(j, parts=2):
                t = w0pool.tile([P, QW], F16, tag="w0", name=f"w0_{j}")
                split_dma(nc.sync, t, wP[j * P:(j + 1) * P, 0:QW], parts)
                w0.append(t)

            # first loads: issue from both queues in parallel
            xa0 = xapool.tile([P, XAW], F16, tag="xa", name="xa_0")
            dual_dma(xa0, xaT[0:P, :], 4)
            xs.append(xa0)
            w00 = w0pool.tile([P, QW], F16, tag="w0", name="w0_0")
            dual_dma(w00, wP[0:P, 0:QW], 4)
            w0.append(w00)
            xa1 = xapool.tile([P, XAW], F16, tag="xa", name="xa_1")
            dual_dma(xa1, xaT[P:2 * P, :], 2)
            xs.append(xa1)
            w01 = w0pool.tile([P, QW], F16, tag="w0", name="w0_1")
            dual_dma(w01, wP[P:2 * P, 0:QW], 4)
            w0.append(w01)
            bss = cpool.tile([r_aug, d_out], F16, tag="bss", name="bss")
            mss = cpool.tile([r_aug, tpc], F32, tag="mss", name="mss")
            xam = cpool.tile([r_aug, tpc], F16, tag="xam", name="xam")
            dummy = cpool.tile([P, NFREE], F16, tag="dummy", name="dummy")
            for k in range(2, KT):
                xa_dma(k)
                j = k // QK + 1  # quad prefetch ~1 quad ahead of use
                if k % QK == 0 and j < NQ:
                    w0_dma(j)
                if k == 20:
                    # small mid-flight loads, only needed at mask time
                    # (issued late so they don't delay the startup slabs)
                    split_dma(nc.sync, bss, bS, 2)
                    split_dma(nc.sync, mss, mS, 2)

            # warm-up: garbage matmuls with no input DMA dependency keep
            # the PE busy while the first slab lands and finish the ~3 us
            # p-state ramp before real work arrives (shares the psum slot
            # rotation, so a later tile's first write waits for them — they
            # are long done by then)
            nc.vector.memset(dummy, 0.0)
            warm_ps = psum.tile([P, NFREE], F32, tag="ps", name="warm_ps")
            for i in range(12):
                nc.tensor.matmul(warm_ps, lhsT=dummy[:, 0:P],
                                 rhs=dummy, start=True, stop=True)

            # ones row for the bias contraction (mss row 64 is all 1.0)
            nc.vector.tensor_copy(xam[MAX_RANK:r_aug, :],
                                  mss[MAX_RANK:r_aug, :])

            # ---- PE startup: per k, LoRA-A + 6 inline n=0 m-tiles ----
            lps = psum.tile([P, NFREE], F32, tag="ps", name="ps_lora")
            lora_ps = [lps[c * MAX_RANK:(c + 1) * MAX_RANK, :]
                       for c in range(t_chunks)]
            psts0 = [psum.tile([P, NFREE], F32, tag="ps", name=f"pst0_{m}")
                     for m in range(m_inline)]
            for k in range(KT):
                wq = w0[k // QK]
                qs = (k % QK) * NFREE
                for c in range(t_chunks):
                    nc.tensor.matmul(
                        lora_ps[c],
                        lhsT=xs[k][:, tpc:tpc + MAX_RANK],
                        rhs=xs[k][:, c * NFREE:(c + 1) * NFREE],
                        start=(k == 0),
                        stop=(k == KT - 1),
                    )
                for m in range(m_inline):
                    nc.tensor.matmul(
                        psts0[m],
                        lhsT=xs[k][:, m * P:(m + 1) * P],
                        rhs=wq[:, qs:qs + NFREE],
                        start=(k == 0),
                        stop=False,
                    )

            # ---- masks: xam = lora_ps * mS (frees the two LoRA banks) ----
            for c in range(t_chunks):
                tsl = slice(c * NFREE, (c + 1) * NFREE)
                nc.vector.tensor_mul(xam[0:MAX_RANK, tsl],
                                     lora_ps[c],
                                     mss[0:MAX_RANK, tsl])

            def lora_b(pst, m, nsl):
                nc.tensor.matmul(
                    pst,
                    lhsT=xam[:, m * P:(m + 1) * P],
                    rhs=bss[:, nsl],
                    start=False,
                    stop=True,
                )

            def copy_out(m, n, pst, parts=2, eng=None):
                ot = opool.tile([P, NFREE], F32, tag="ot", name=f"ot_{n}_{m}")
                nc.vector.tensor_copy(ot, pst)
                # scalar engine's DGE queue, split so rows land on several
                # queues (a single 256 KiB store drains at ~19 GB/s)
                rows = P // parts
                for i in range(parts):
                    (eng or nc.scalar).dma_start(
                        out[m * P + i * rows:m * P + (i + 1) * rows,
                            n * NFREE:(n + 1) * NFREE],
                        ot[i * rows:(i + 1) * rows, :])

            # ---- n=0 completion for the inline m-tiles ----
            n0sl = slice(0, NFREE)
            for m in range(m_inline):
                lora_b(psts0[m], m, n0sl)
                copy_out(m, 0, psts0[m])

            # ---- steady state: n = 1..7 ----
            for n in range(1, NT):
                nsl = slice(n * NFREE, (n + 1) * NFREE)
                psts = [psum.tile([P, NFREE], F32, tag="ps",
                                  name=f"pst_{n}_{m}") for m in range(MT)]
                for j in range(NQ):
                    wq = wpool.tile([P, QW], F16, tag="wt", name=f"wt_{n}_{j}")
                    split_dma(nc.sync, wq,
                              wP[j * P:(j + 1) * P, n * QW:(n + 1) * QW], 2)
                    for q in range(QK):
                        k = j * QK + q
                        for m in range(MT):
                            nc.tensor.matmul(
                                psts[m],
                                lhsT=xs[k][:, m * P:(m + 1) * P],
                                rhs=wq[:, q * NFREE:(q + 1) * NFREE],
                                start=(k == 0),
                                stop=False,
                            )
                            if k == KT - 1:
                                # finish tile m right away: the psum bank
                                # frees while the PE streams m+1..7, so the
                                # next n-block starts without a copy stall
                                lora_b(psts[m], m, nsl)
                                copy_out(m, n, psts[m])

            # ---- tail: the two n=0 m-tiles displaced by the LoRA-A psum
            # accumulators, straight from the resident quads (no DMA in,
            # only 512 KiB out -> short drain) ----
            pstd = [psum.tile([P, NFREE], F32, tag="ps", name=f"pstd_{m}")
                    for m in defer]
            for k in range(KT):
                wq = w0[k // QK]
                qs = (k % QK) * NFREE
                for i, m in enumerate(defer):
                    nc.tensor.matmul(
                        pstd[i],
                        lhsT=xs[k][:, m * P:(m + 1) * P],
                        rhs=wq[:, qs:qs + NFREE],
                        start=(k == 0),
                        stop=False,
                    )
                    if k == KT - 1:
                        lora_b(pstd[i], m, n0sl)
                        # final stores: 4-way splits, issue queues spread
                        # over two engines so the drain parallelizes
                        copy_out(m, 0, pstd[i], parts=4,
                                 eng=nc.sync if i == 0 else nc.scalar)

    nc.compile()
    return nc


def _prep_core_inputs(x16, weight_p, bias, a_cache, b_cache, tok_adapter,
                      tok_scale, rank_page_table, ranks, core):
    """Host-side shard prep for one core (single adapter per core)."""
    d_out = b_cache.shape[1]
    r_aug = MAX_RANK + 1
    sl = slice(core * TPC, (core + 1) * TPC)
    adapters = tok_adapter[sl]
    scales = tok_scale[sl]
    a = int(adapters[0])
    pages = rank_page_table[a]  # [64] page ids

    xaT = np.empty((D_IN, XAW), np.float16)
    xaT[:, :TPC] = x16[sl].T
    xaT[:, TPC:] = a_cache[pages].T.astype(np.float16)

    bS = np.empty((r_aug, d_out), np.float16)
    bS[:MAX_RANK] = b_cache[pages].astype(np.float16)
    bS[MAX_RANK] = bias.astype(np.float16)

    mS = np.empty((r_aug, TPC), np.float32)
    slot_active = (np.arange(MAX_RANK) < ranks[a])[:, None]  # [64, 1]
    tok_active = (adapters == a)[None, :]  # [1, TPC]
    mS[:MAX_RANK] = (slot_active & tok_active) * scales[None, :]
    mS[MAX_RANK] = 1.0
    return {"xaT": xaT, "wP": weight_p, "bS": bS, "mS": mS}


def kernel(x, weight, bias, a_cache, b_cache, b_start_loc, b_adapter_ids,
           b_scaling, rank_page_table, ranks):
    global last_exec_time_ns, last_results
    x = np.asarray(x, np.float32)
    weight = np.asarray(weight, np.float32)
    bias = np.asarray(bias, np.float32)
    a_cache = np.asarray(a_cache, np.float32)
    b_cache = np.asarray(b_cache, np.float32)
    b_start_loc = np.asarray(b_start_loc)
    b_adapter_ids = np.asarray(b_adapter_ids)
    b_scaling = np.asarray(b_scaling, np.float32)
    rank_page_table = np.asarray(rank_page_table)
    ranks = np.asarray(ranks)

    t = x.shape[0]
    seg = np.searchsorted(b_start_loc, np.arange(t, dtype=b_start_loc.dtype),
                          side="right") - 1
    tok_adapter = b_adapter_ids[seg]
    tok_scale = b_scaling[seg]
    # this schedule assumes one adapter per 1024-token core slice (the
    # spec's equal-length-sequence layout guarantees it)
    assert all(
        len(np.unique(tok_adapter[c * TPC:(c + 1) * TPC])) == 1
        for c in range(N_CORES)
    )

    x16 = x.astype(np.float16)
    # pack W^T into [NQ*128, NT*2048]: row j*128+p, col n*2048 + q*512 + c
    # holds W^T[(4j+q)*128 + p, n*512 + c]
    wt = np.ascontiguousarray(weight.T.astype(np.float16))
    weight_p = np.ascontiguousarray(
        wt.reshape(NQ, QK, P, NT, NFREE)
          .transpose(0, 2, 3, 1, 4)
          .reshape(NQ * P, NT * QK * NFREE))

    in_maps = [
        _prep_core_inputs(x16, weight_p, bias, a_cache, b_cache, tok_adapter,
                          tok_scale, rank_page_table, ranks, c)
        for c in range(N_CORES)
    ]

    nc = _build_program()
    trace = os.environ.get("KERNEL_TRACE", "0") == "1"
    repeat = int(os.environ.get("KERNEL_REPEAT", "1"))
    times = []
    for _ in range(repeat):
        res = run_bass_kernel_spmd(nc, in_maps, core_ids=list(range(N_CORES)),
                                   trace=trace)
        times.append(res.exec_time_ns)
    last_exec_time_ns = (min(t for t in times if t is not None)
                         if any(t is not None for t in times) else None)
    last_results = res
    if repeat > 1:
        print("exec times:", times)
    return np.concatenate([res.results[c]["out"] for c in range(N_CORES)],
                          axis=0).astype(np.float32)
